# revision 13
# baseline (speedup 1.0000x reference)
"""ContextualAttentionMask Trainium2 kernel (fp8 DoubleRow version).

Math (per batch sample):
  f: [256, 4096] feature map (channels x pixels), m: [4096] mask
  K[j, :]    = f[:, j] + 1e-7          (per-pixel 1x1 kernel)
  rstd[j]    = 1 / ||K[j, :]||_2
  raw[j, n]  = sum_c f[c, j] * f[c, n]          (only interior columns matter:
               the conv padding columns are dead compute - 1x1 kernels, the
               output at pad positions is cropped, softmax is per-column)
  att[j, n]  = softmax_j(rstd[j] * raw[j, n])
  fmap[c, n] = sum_j rstd[j] * m[j] * K[j, c] * att[j, n]
  final      = fmap * (1 - m) + f * m  ;  skip branch if mask nearly all-ones

Device computes (per core, unnormalized; host divides, blends, skip-branch):
  E[j, n] = exp(fs[:, j] . f[:, n] + ebias)   with fs = fp8(rstd * K) so the
            per-row rstd scale is folded into the GEMM1 stationary operand;
            ebias = 5 - max_n ||f_n|| keeps E inside fp8-e4m3 range (max 240)
            and cancels in the host-side division.
  o[c, n] = sum_j km8[j, c] * E[j, n]     with km8 = fp8(rstd * m * K)
  s[n]    = sum_j E[j, n]                 (fp8 ones-matmuls on the PE)

All three matmul streams run as fp8 DoubleRow (2 contraction rows per PE
cell): GEMM1 contracts ch=256 as one 2x128 instruction per j-block; GEMM2 and
the ones-sum contract j in pair-groups of 2x128. The exp reads two PSUM banks
per instruction (both j-blocks of a pair-group) so the fixed activation
overhead is amortized over 1024 elements.

Sharding: 8 cores = 4 samples x 2 column-halves (2048 columns each). fs8/km8
are j-indexed (full range, identical for both halves); f8 holds only the
core's own 2048 columns. Host-side prep: +1e-7, rstd, fp8 casts, layouts.
"""

import sys
from contextlib import ExitStack

import numpy as np

sys.path.insert(0, "/opt/trn_rl_repo")

from concourse import bacc, mybir, tile  # noqa: E402
from concourse.bass_utils import run_bass_kernel_spmd  # noqa: E402

FP32 = mybir.dt.float32
FP16 = mybir.dt.float16
FP8 = mybir.dt.float8e4
DR = mybir.MatmulPerfMode.DoubleRow

CH = 256          # channels
J = 4096          # number of per-pixel kernels (= h*w)
NH = 2048         # columns handled per core (half of a sample)


def build_program(ch=CH, j_total=J, n_half=NH, bufs_e=2, loop_reps=1, lag=1,
                  sum_mode="pe", act_b=3):
    """Emit the per-core Bass/Tile program (SPMD across 8 cores)."""
    assert ch == 256 and j_total % 256 == 0
    n_jb = j_total // 128     # j blocks
    n_g = j_total // 256      # j pair-groups (two 128-blocks each)
    qs = min(512, n_half)     # output column chunk width
    nq = n_half // qs
    assert n_half % qs == 0
    # activation batches: act_b j-blocks per exp instruction (PSUM limit:
    # 2 double-buffered score tiles of act_b banks + 2 GEMM2 banks <= 8)
    act_groups = []
    a = 0
    while a < n_jb:
        act_groups.append((a, min(act_b, n_jb - a)))
        a += act_b

    nc = bacc.Bacc("TRN2", target_bir_lowering=False, debug=False, num_devices=8)

    fs8_d = nc.dram_tensor("fs8", [128, 2 * j_total], FP8, kind="ExternalInput").ap()
    f8_d = nc.dram_tensor("f8", [128, 2 * n_half], FP8, kind="ExternalInput").ap()
    km8_d = nc.dram_tensor("km8", [128, 2 * j_total], FP8, kind="ExternalInput").ap()
    eb_d = nc.dram_tensor("ebias", [128, 1], FP32, kind="ExternalInput").ap()
    o_d = nc.dram_tensor("o", [128, 2 * n_half], FP16, kind="ExternalOutput").ap()
    s_d = nc.dram_tensor("s", [1, n_half], FP32, kind="ExternalOutput").ap()

    with tile.TileContext(nc) as tc, ExitStack() as ctx:
        const_p = ctx.enter_context(tc.tile_pool(name="const", bufs=1))
        in_p = ctx.enter_context(tc.tile_pool(name="inp", bufs=1))
        e_p = ctx.enter_context(tc.tile_pool(name="e", bufs=bufs_e))
        osb_p = ctx.enter_context(tc.tile_pool(name="osb", bufs=3))
        ssb_p = ctx.enter_context(tc.tile_pool(name="ssb", bufs=2))
        ps_sc = ctx.enter_context(tc.tile_pool(name="ps_sc", bufs=2, space="PSUM"))
        ps_out = ctx.enter_context(tc.tile_pool(name="ps_out", bufs=2, space="PSUM"))

        # dual-fp8 ldweights needs >= 16 stationary columns; 16 duplicate
        # sum rows cost the same moving time, host reads row 0.
        ones8 = const_p.tile([128, 2, 16], FP8)
        nc.vector.memset(ones8[:], 1.0)
        ebias = const_p.tile([128, 1], FP32, tag="ebias")

        # fp8 inputs. fs8 = rstd-scaled kernels [c%128, c//128, j] (GEMM1
        # stationary); f8 = own columns [c%128, c//128, n] (GEMM1 moving);
        # km8 = mask*rstd-scaled kernels [j%128, group, pair, c].
        fs8 = in_p.tile([128, 2, j_total], FP8, tag="fs8")
        f8 = in_p.tile([128, 2, n_half], FP8, tag="f8")
        km8 = in_p.tile([128, n_g, 2, ch], FP8, tag="km8")

        # DMA order: first-needed first. ebias rides first (needed by the
        # first exp); fs8 j-chunks and f8's first chunk feed the first
        # matmuls; km8 groups are needed once the first exp completes.
        nc.sync.dma_start(out=ebias[:], in_=eb_d[:, :])
        for t in range(2):
            nc.sync.dma_start(out=f8[:, t, 0:qs], in_=f8_d[:, t * n_half:t * n_half + qs])
        jb_bounds = [0, 512, 1024, 2048, j_total]
        for a, b in zip(jb_bounds[:-1], jb_bounds[1:]):
            for t in range(2):
                nc.sync.dma_start(
                    out=fs8[:, t, a:b], in_=fs8_d[:, t * j_total + a:t * j_total + b]
                )
        for a, b in zip(jb_bounds[:-1], jb_bounds[1:]):
            ga, gb = a // 256, b // 256
            nc.sync.dma_start(
                out=km8[:, ga:gb, :, :], in_=km8_d[:, a * 2:b * 2]
            )
        for t in range(2):
            if n_half > qs:
                nc.sync.dma_start(
                    out=f8[:, t, qs:n_half],
                    in_=f8_d[:, t * n_half + qs:(t + 1) * n_half],
                )

        # fused main loop: scores -> exp -> Km^T E, then ones^T E at the
        # chunk tail (after the GEMM2 banks free up; the e mega-tile keeps
        # every group's exp output alive through the chunk).  Activation
        # batching (act_b j-blocks per exp) is decoupled from GEMM2's
        # DoubleRow pairing via subtile dependencies on the e mega-tile.
        # loop_reps > 1 repeats the identical work (timing experiments only).
        for q in [qq for _ in range(loop_reps) for qq in range(nq)]:
            nsl = slice(q * qs, (q + 1) * qs)
            out_ps = [
                ps_out.tile([128, qs], FP32, tag="out", name=f"out_ps{cb}")
                for cb in range(2)
            ]
            e_all = e_p.tile([128, n_jb, qs], FP8, tag="e", name="e_all")

            # software pipeline: GEMM2 pairs trail the exp stream by `lag`
            # activation groups so the in-order PE queue never waits on ACT.
            h_next = 0

            def emit_pairs(avail_jb):
                nonlocal h_next
                while h_next < n_g and 2 * h_next + 2 <= avail_jb:
                    h = h_next
                    for cb in range(2):
                        nc.tensor.matmul(
                            out_ps[cb][:],
                            km8[:, h, :, cb * 128:(cb + 1) * 128],
                            e_all[:, 2 * h:2 * h + 2, :],
                            start=(h == 0), stop=(h == n_g - 1), perf_mode=DR,
                        )
                    h_next += 1

            for ai, (a0, cnt) in enumerate(act_groups):
                ps3 = ps_sc.tile([128, act_b, qs], FP32, tag="sc", name="ps3")
                for i in range(cnt):
                    jb = a0 + i
                    nc.tensor.matmul(
                        ps3[:, i, :],
                        fs8[:, :, jb * 128:(jb + 1) * 128],
                        f8[:, :, nsl],
                        start=True, stop=True, perf_mode=DR,
                    )
                nc.scalar.activation(
                    e_all[:, a0:a0 + cnt, :], ps3[:, 0:cnt, :],
                    mybir.ActivationFunctionType.Exp,
                    bias=ebias[:], scale=1.0,
                )
                done = ai + 1 - lag
                if done >= 1:
                    avail = act_groups[done - 1][0] + act_groups[done - 1][1]
                    emit_pairs(avail)
            emit_pairs(n_jb)

            for cb in range(2):
                osb = osb_p.tile([128, qs], FP16, tag="osb", name="osb")
                nc.vector.tensor_copy(osb[:], out_ps[cb][:])
                nc.sync.dma_start(
                    out=o_d[:, cb * n_half + q * qs:cb * n_half + (q + 1) * qs],
                    in_=osb[:],
                )
            srow = ssb_p.tile([1, qs], FP32, tag="srow", name="srow")
            if sum_mode == "pe":
                sum_ps = ps_out.tile([16, qs], FP32, tag="out", name="sum_ps")
                for h in range(n_g):
                    nc.tensor.matmul(
                        sum_ps[:], ones8[:, :, :], e_all[:, 2 * h:2 * h + 2, :],
                        start=(h == 0), stop=(h == n_g - 1), perf_mode=DR,
                    )
                nc.vector.tensor_copy(srow[:], sum_ps[0:1, :])
            else:  # timing-only variant: output garbage sums
                nc.vector.memset(srow[:], 1.0)
            nc.sync.dma_start(out=s_d[0:1, nsl], in_=srow[:])

    nc.compile()
    return nc


_CACHE = {}


def _get_program():
    if "nc" not in _CACHE:
        _CACHE["nc"] = build_program()
    return _CACHE["nc"]


def _get_runner():
    """Cached sharded executable over 8 cores (same program/plugin as
    run_bass_kernel_spmd's axon path, but without per-call retracing)."""
    if "runner" in _CACHE:
        return _CACHE["runner"]
    import jax
    from jax.sharding import Mesh, NamedSharding, PartitionSpec
    from jax.experimental.shard_map import shard_map
    from concourse import bass2jax, mybir
    from concourse.bass2jax import _bass_exec_p, partition_id_tensor

    nc = _get_program()
    bass2jax.install_neuronx_cc_hook()
    pname = nc.partition_id_tensor.name if nc.partition_id_tensor else None

    in_names, out_names, out_avals = [], [], []
    for alloc in nc.m.functions[0].allocations:
        if not isinstance(alloc, mybir.MemoryLocationSet):
            continue
        name = alloc.memorylocations[0].name
        if alloc.kind == "ExternalInput":
            if name != pname:
                in_names.append(name)
        elif alloc.kind == "ExternalOutput":
            out_names.append(name)
            out_avals.append(
                jax.core.ShapedArray(
                    tuple(alloc.tensor_shape), mybir.dt.np(alloc.dtype)
                )
            )
    n_params, n_outs = len(in_names), len(out_names)
    all_in = in_names + out_names + ([pname] if pname else [])

    def _body(*args):
        operands = list(args)
        if pname is not None:
            operands.append(partition_id_tensor())
        return tuple(_bass_exec_p.bind(
            *operands, out_avals=tuple(out_avals), in_names=tuple(all_in),
            out_names=tuple(out_names), lowering_input_output_aliases=(),
            sim_require_finite=True, sim_require_nnan=True, nc=nc,
        ))

    devices = jax.devices()[:8]
    mesh = Mesh(np.asarray(devices), ("core",))
    spec = NamedSharding(mesh, PartitionSpec("core"))
    fn = jax.jit(
        shard_map(
            _body, mesh=mesh,
            in_specs=(PartitionSpec("core"),) * (n_params + n_outs),
            out_specs=(PartitionSpec("core"),) * n_outs,
            check_rep=False,
        ),
        donate_argnums=tuple(range(n_params, n_params + n_outs)),
        keep_unused=True,
    )
    zero_host = [
        np.zeros((8 * a.shape[0], *a.shape[1:]), a.dtype) for a in out_avals
    ]

    def run(in_maps):
        concat_in = [
            np.concatenate([np.asarray(m[name]) for m in in_maps], axis=0)
            for name in in_names
        ]
        zeros = [jax.device_put(z, spec) for z in zero_host]
        out = fn(*concat_in, *zeros)
        return [
            {
                name: np.asarray(out[i]).reshape(8, *out_avals[i].shape)[c]
                for i, name in enumerate(out_names)
            }
            for c in range(8)
        ]

    _CACHE["runner"] = run
    return run


def make_in_maps(foreground, mask):
    """Per-core host-side input prep (fp8 casts + device layouts)."""
    import ml_dtypes
    F8 = ml_dtypes.float8_e4m3

    bs, ch, h, w = foreground.shape
    hw = h * w
    half = hw // 2
    f = np.ascontiguousarray(foreground.reshape(bs, ch, hw), dtype=np.float32)
    m = np.ascontiguousarray(mask.reshape(bs, hw), dtype=np.float32)
    in_maps = []
    for b in range(bs):
        k = f[b] + np.float32(1e-7)                 # [ch, hw], reference's +1e-7
        rstd = 1.0 / np.sqrt((k * k).sum(axis=0, dtype=np.float64))  # [hw]
        rstd = rstd.astype(np.float32)
        # [c, j] -> [c%128, c//128, j] -> [128, 2*hw]
        f8_full = f[b].astype(F8).reshape(2, 128, hw).transpose(1, 0, 2)
        fs8 = (k * rstd[None, :]).astype(F8).reshape(2, 128, hw)
        fs8 = np.ascontiguousarray(fs8.transpose(1, 0, 2)).reshape(128, 2 * hw)
        # [j, c] -> [j%128, j//256, (j//128)%2, c] -> [128, 2*hw]
        km = ((rstd * m[b])[:, None] * k.T).astype(F8)  # [hw, ch]
        km8 = np.ascontiguousarray(
            km.reshape(hw // 256, 2, 128, ch).transpose(2, 0, 1, 3)
        ).reshape(128, 2 * hw)
        norm = (1.0 / rstd)
        for hh in range(2):
            cols = slice(hh * half, (hh + 1) * half)
            f8c = np.ascontiguousarray(f8_full[:, :, cols]).reshape(128, 2 * half)
            eb = np.full((128, 1), 5.0 - norm[cols].max(), dtype=np.float32)
            in_maps.append({
                "fs8": fs8, "f8": f8c, "km8": km8, "ebias": eb,
            })
    return in_maps


def kernel(foreground, mask):
    foreground = np.asarray(foreground, dtype=np.float32)
    mask = np.asarray(mask, dtype=np.float32)
    bs, ch, h, w = foreground.shape
    hw = h * w

    in_maps = make_in_maps(foreground, mask)
    try:
        results = _get_runner()(in_maps)
    except Exception:
        # robust fallback: the generic SPMD entry point
        res = run_bass_kernel_spmd(_get_program(), in_maps, list(range(8)))
        results = res.results

    fmap = np.empty((bs, ch, h, w), dtype=np.float32)
    rows = h // 2
    for core in range(8):
        b, hh = core // 2, core % 2
        o = results[core]["o"]       # [128, 2*hw/2] fp16 unnormalized
        s = results[core]["s"]       # [1, hw/2] softmax denominator
        o_f = o.astype(np.float32).reshape(128, 2, hw // 2)
        o_f = o_f.transpose(1, 0, 2).reshape(ch, hw // 2)
        fmap[b, :, hh * rows:(hh + 1) * rows, :] = (o_f / s).reshape(ch, rows, w)

    mm = mask[:, 0:1]                    # [bs, 1, h, w]
    final = fmap * (1.0 - mm) + foreground * mm
    skip = mask.sum(axis=(1, 2, 3)) > (hw - 10)
    final[skip] = foreground[skip]
    return final.astype(np.float32)


# revision 16
# speedup vs baseline: 1.3087x; 1.3087x over previous
"""ContextualAttentionMask Trainium2 kernel (fp8 DoubleRow version).

Math (per batch sample):
  f: [256, 4096] feature map (channels x pixels), m: [4096] mask
  K[j, :]    = f[:, j] + 1e-7          (per-pixel 1x1 kernel)
  rstd[j]    = 1 / ||K[j, :]||_2
  raw[j, n]  = sum_c f[c, j] * f[c, n]          (only interior columns matter:
               the conv padding columns are dead compute - 1x1 kernels, the
               output at pad positions is cropped, softmax is per-column)
  att[j, n]  = softmax_j(rstd[j] * raw[j, n])
  fmap[c, n] = sum_j rstd[j] * m[j] * K[j, c] * att[j, n]
  final      = fmap * (1 - m) + f * m  ;  skip branch if mask nearly all-ones

Device computes (per core, unnormalized; host divides, blends, skip-branch):
  E[j, n] = exp(fs[:, j] . f[:, n] + ebias)   with fs = fp8(rstd * K) so the
            per-row rstd scale is folded into the GEMM1 stationary operand;
            ebias = 5 - max_n ||f_n|| keeps E inside fp8-e4m3 range (max 240)
            and cancels in the host-side division.
  o[c, n] = sum_j km8[j, c] * E[j, n]     with km8 = fp8(rstd * m * K)
  s[n]    = sum_j E[j, n]                 (fp8 ones-matmuls on the PE)

All three matmul streams run as fp8 DoubleRow (2 contraction rows per PE
cell): GEMM1 contracts ch=256 as one 2x128 instruction per j-block; GEMM2 and
the ones-sum contract j in pair-groups of 2x128. The exp reads two PSUM banks
per instruction (both j-blocks of a pair-group) so the fixed activation
overhead is amortized over 1024 elements.

Sharding: 8 cores = 4 samples x 2 column-halves (2048 columns each). fs8/km8
are j-indexed (full range, identical for both halves); f8 holds only the
core's own 2048 columns. Host-side prep: +1e-7, rstd, fp8 casts, layouts.
"""

import sys
from contextlib import ExitStack

import numpy as np

sys.path.insert(0, "/opt/trn_rl_repo")

from concourse import bacc, mybir, tile  # noqa: E402
from concourse.bass_utils import run_bass_kernel_spmd  # noqa: E402

FP32 = mybir.dt.float32
FP16 = mybir.dt.float16
FP8 = mybir.dt.float8e4
DR = mybir.MatmulPerfMode.DoubleRow

CH = 256          # channels
J = 4096          # number of per-pixel kernels (= h*w)
NH = 2048         # columns handled per core (half of a sample)


def build_program(ch=CH, j_total=J, n_half=NH, bufs_e=2, loop_reps=1, lag=2,
                  sum_mode="pe", act_b=3):
    """Emit the per-core Bass/Tile program (SPMD across 8 cores)."""
    assert ch == 256 and j_total % 256 == 0
    n_jb = j_total // 128     # j blocks
    n_g = j_total // 256      # j pair-groups (two 128-blocks each)
    qs = min(512, n_half)     # output column chunk width
    nq = n_half // qs
    assert n_half % qs == 0
    # activation batches: act_b j-blocks per exp instruction (PSUM limit:
    # 2 double-buffered score tiles of act_b banks + 2 GEMM2 banks <= 8)
    act_groups = []
    a = 0
    while a < n_jb:
        act_groups.append((a, min(act_b, n_jb - a)))
        a += act_b

    nc = bacc.Bacc("TRN2", target_bir_lowering=False, debug=False, num_devices=8)

    fs8_d = nc.dram_tensor("fs8", [128, 2 * j_total], FP8, kind="ExternalInput").ap()
    f8_d = nc.dram_tensor("f8", [128, 2 * n_half], FP8, kind="ExternalInput").ap()
    km8_d = nc.dram_tensor("km8", [128, 2 * j_total], FP8, kind="ExternalInput").ap()
    eb_d = nc.dram_tensor("ebias", [128, 1], FP32, kind="ExternalInput").ap()
    o_d = nc.dram_tensor("o", [128, 2 * n_half], FP16, kind="ExternalOutput").ap()
    s_d = nc.dram_tensor("s", [1, n_half], FP32, kind="ExternalOutput").ap()

    with tile.TileContext(nc) as tc, ExitStack() as ctx:
        const_p = ctx.enter_context(tc.tile_pool(name="const", bufs=1))
        in_p = ctx.enter_context(tc.tile_pool(name="inp", bufs=1))
        e_p = ctx.enter_context(tc.tile_pool(name="e", bufs=bufs_e))
        osb_p = ctx.enter_context(tc.tile_pool(name="osb", bufs=3))
        ssb_p = ctx.enter_context(tc.tile_pool(name="ssb", bufs=2))
        ps_sc = ctx.enter_context(tc.tile_pool(name="ps_sc", bufs=2, space="PSUM"))
        ps_out = ctx.enter_context(tc.tile_pool(name="ps_out", bufs=2, space="PSUM"))

        # dual-fp8 ldweights needs >= 16 stationary columns; 16 duplicate
        # sum rows cost the same moving time, host reads row 0.
        ones8 = const_p.tile([128, 2, 16], FP8)
        nc.vector.memset(ones8[:], 1.0)
        ebias = const_p.tile([128, 1], FP32, tag="ebias")

        # fp8 inputs. fs8 = rstd-scaled kernels [c%128, c//128, j] (GEMM1
        # stationary); f8 = own columns [c%128, c//128, n] (GEMM1 moving);
        # km8 = mask*rstd-scaled kernels [j%128, group, pair, c].
        fs8 = in_p.tile([128, 2, j_total], FP8, tag="fs8")
        f8 = in_p.tile([128, 2, n_half], FP8, tag="f8")
        km8 = in_p.tile([128, n_g, 2, ch], FP8, tag="km8")

        # DMA order: first-needed first. ebias rides first (needed by the
        # first exp); fs8 j-chunks and f8's first chunk feed the first
        # matmuls; km8 groups are needed once the first exp completes.
        nc.sync.dma_start(out=ebias[:], in_=eb_d[:, :])
        for t in range(2):
            nc.sync.dma_start(out=f8[:, t, 0:qs], in_=f8_d[:, t * n_half:t * n_half + qs])
        jb_bounds = [0, 512, 1024, 2048, j_total]
        for a, b in zip(jb_bounds[:-1], jb_bounds[1:]):
            for t in range(2):
                nc.sync.dma_start(
                    out=fs8[:, t, a:b], in_=fs8_d[:, t * j_total + a:t * j_total + b]
                )
        for a, b in zip(jb_bounds[:-1], jb_bounds[1:]):
            ga, gb = a // 256, b // 256
            nc.sync.dma_start(
                out=km8[:, ga:gb, :, :], in_=km8_d[:, a * 2:b * 2]
            )
        for t in range(2):
            if n_half > qs:
                nc.sync.dma_start(
                    out=f8[:, t, qs:n_half],
                    in_=f8_d[:, t * n_half + qs:(t + 1) * n_half],
                )

        # fused main loop: scores -> exp -> Km^T E per chunk. The ones^T E
        # softmax denominators for chunk q are deferred into chunk q+1's
        # first two activation-group slots: the GEMM2 PSUM banks are free
        # by then, the PE has idle room there (it is ACT-paced), and the
        # ACT engine never starves at a chunk boundary. The e mega-tile
        # (bufs=2) keeps chunk q's exps alive through chunk q+1's sums.
        # Activation batching (act_b j-blocks per exp) is decoupled from
        # GEMM2's DoubleRow pairing via subtile deps on the e mega-tile.
        # loop_reps > 1 repeats the identical work (timing experiments only).
        def flush_sums(e_prev, q_prev, h0, h1, sum_ps):
            for h in range(h0, h1):
                nc.tensor.matmul(
                    sum_ps[:], ones8[:, :, :], e_prev[:, 2 * h:2 * h + 2, :],
                    start=(h == 0), stop=(h == n_g - 1), perf_mode=DR,
                )
            if h1 == n_g:
                srow = ssb_p.tile([1, qs], FP32, tag="srow", name="srow")
                nc.vector.tensor_copy(srow[:], sum_ps[0:1, :])
                nc.sync.dma_start(
                    out=s_d[0:1, q_prev * qs:(q_prev + 1) * qs], in_=srow[:]
                )

        pending = None  # (e_all, q) of the previous chunk, sums not yet done
        for q in [qq for _ in range(loop_reps) for qq in range(nq)]:
            nsl = slice(q * qs, (q + 1) * qs)
            # allocation order matters: sum_ps must take the bank freed by
            # the previous chunk's out_ps0 BEFORE this chunk's accumulators
            # claim slots, or the bufs=2 rotation serializes the chunk.
            if pending is not None:
                sum_ps = ps_out.tile([16, qs], FP32, tag="out", name="sum_ps")
            out_ps = [
                ps_out.tile([128, qs], FP32, tag="out", name=f"out_ps{cb}")
                for cb in range(2)
            ]
            e_all = e_p.tile([128, n_jb, qs], FP8, tag="e", name="e_all")

            # software pipeline: GEMM2 pairs trail the exp stream by `lag`
            # activation groups so the in-order PE queue never waits on ACT.
            h_next = 0

            def emit_pairs(avail_jb):
                nonlocal h_next
                while h_next < n_g and 2 * h_next + 2 <= avail_jb:
                    h = h_next
                    for cb in range(2):
                        nc.tensor.matmul(
                            out_ps[cb][:],
                            km8[:, h, :, cb * 128:(cb + 1) * 128],
                            e_all[:, 2 * h:2 * h + 2, :],
                            start=(h == 0), stop=(h == n_g - 1), perf_mode=DR,
                        )
                    h_next += 1

            for ai, (a0, cnt) in enumerate(act_groups):
                ps3 = ps_sc.tile([128, act_b, qs], FP32, tag="sc", name="ps3")
                for i in range(cnt):
                    jb = a0 + i
                    nc.tensor.matmul(
                        ps3[:, i, :],
                        fs8[:, :, jb * 128:(jb + 1) * 128],
                        f8[:, :, nsl],
                        start=True, stop=True, perf_mode=DR,
                    )
                if pending is not None and ai < 2:
                    flush_sums(pending[0], pending[1],
                               ai * (n_g // 2), (ai + 1) * (n_g // 2), sum_ps)
                    if ai == 1:
                        pending = None
                nc.scalar.activation(
                    e_all[:, a0:a0 + cnt, :], ps3[:, 0:cnt, :],
                    mybir.ActivationFunctionType.Exp,
                    bias=ebias[:], scale=1.0,
                )
                done = ai + 1 - lag
                if done >= 1:
                    avail = act_groups[done - 1][0] + act_groups[done - 1][1]
                    emit_pairs(avail)
            emit_pairs(n_jb)

            for cb in range(2):
                osb = osb_p.tile([128, qs], FP16, tag="osb", name="osb")
                nc.vector.tensor_copy(osb[:], out_ps[cb][:])
                nc.sync.dma_start(
                    out=o_d[:, cb * n_half + q * qs:cb * n_half + (q + 1) * qs],
                    in_=osb[:],
                )
            if sum_mode == "pe":
                pending = (e_all, q)
            else:  # timing-only variant: output garbage sums
                srow = ssb_p.tile([1, qs], FP32, tag="srow", name="srow")
                nc.vector.memset(srow[:], 1.0)
                nc.sync.dma_start(out=s_d[0:1, nsl], in_=srow[:])
        if pending is not None:  # tail: last chunk's sums
            sum_ps = ps_out.tile([16, qs], FP32, tag="out", name="sum_ps")
            flush_sums(pending[0], pending[1], 0, n_g, sum_ps)

    nc.compile()
    return nc


_CACHE = {}


def _get_program():
    if "nc" not in _CACHE:
        _CACHE["nc"] = build_program()
    return _CACHE["nc"]


def _get_runner():
    """Cached sharded executable over 8 cores (same program/plugin as
    run_bass_kernel_spmd's axon path, but without per-call retracing)."""
    if "runner" in _CACHE:
        return _CACHE["runner"]
    import jax
    from jax.sharding import Mesh, NamedSharding, PartitionSpec
    from jax.experimental.shard_map import shard_map
    from concourse import bass2jax, mybir
    from concourse.bass2jax import _bass_exec_p, partition_id_tensor

    nc = _get_program()
    bass2jax.install_neuronx_cc_hook()
    pname = nc.partition_id_tensor.name if nc.partition_id_tensor else None

    in_names, out_names, out_avals = [], [], []
    for alloc in nc.m.functions[0].allocations:
        if not isinstance(alloc, mybir.MemoryLocationSet):
            continue
        name = alloc.memorylocations[0].name
        if alloc.kind == "ExternalInput":
            if name != pname:
                in_names.append(name)
        elif alloc.kind == "ExternalOutput":
            out_names.append(name)
            out_avals.append(
                jax.core.ShapedArray(
                    tuple(alloc.tensor_shape), mybir.dt.np(alloc.dtype)
                )
            )
    n_params, n_outs = len(in_names), len(out_names)
    all_in = in_names + out_names + ([pname] if pname else [])

    def _body(*args):
        operands = list(args)
        if pname is not None:
            operands.append(partition_id_tensor())
        return tuple(_bass_exec_p.bind(
            *operands, out_avals=tuple(out_avals), in_names=tuple(all_in),
            out_names=tuple(out_names), lowering_input_output_aliases=(),
            sim_require_finite=True, sim_require_nnan=True, nc=nc,
        ))

    devices = jax.devices()[:8]
    mesh = Mesh(np.asarray(devices), ("core",))
    spec = NamedSharding(mesh, PartitionSpec("core"))
    fn = jax.jit(
        shard_map(
            _body, mesh=mesh,
            in_specs=(PartitionSpec("core"),) * (n_params + n_outs),
            out_specs=(PartitionSpec("core"),) * n_outs,
            check_rep=False,
        ),
        donate_argnums=tuple(range(n_params, n_params + n_outs)),
        keep_unused=True,
    )
    zero_host = [
        np.zeros((8 * a.shape[0], *a.shape[1:]), a.dtype) for a in out_avals
    ]

    def run(in_maps):
        concat_in = [
            np.concatenate([np.asarray(m[name]) for m in in_maps], axis=0)
            for name in in_names
        ]
        zeros = [jax.device_put(z, spec) for z in zero_host]
        out = fn(*concat_in, *zeros)
        return [
            {
                name: np.asarray(out[i]).reshape(8, *out_avals[i].shape)[c]
                for i, name in enumerate(out_names)
            }
            for c in range(8)
        ]

    _CACHE["runner"] = run
    return run


def make_in_maps(foreground, mask):
    """Per-core host-side input prep (fp8 casts + device layouts)."""
    import ml_dtypes
    F8 = ml_dtypes.float8_e4m3

    bs, ch, h, w = foreground.shape
    hw = h * w
    half = hw // 2
    f = np.ascontiguousarray(foreground.reshape(bs, ch, hw), dtype=np.float32)
    m = np.ascontiguousarray(mask.reshape(bs, hw), dtype=np.float32)
    in_maps = []
    for b in range(bs):
        k = f[b] + np.float32(1e-7)                 # [ch, hw], reference's +1e-7
        rstd = 1.0 / np.sqrt((k * k).sum(axis=0, dtype=np.float64))  # [hw]
        rstd = rstd.astype(np.float32)
        # [c, j] -> [c%128, c//128, j] -> [128, 2*hw]
        f8_full = f[b].astype(F8).reshape(2, 128, hw).transpose(1, 0, 2)
        fs8 = (k * rstd[None, :]).astype(F8).reshape(2, 128, hw)
        fs8 = np.ascontiguousarray(fs8.transpose(1, 0, 2)).reshape(128, 2 * hw)
        # [j, c] -> [j%128, j//256, (j//128)%2, c] -> [128, 2*hw]
        km = ((rstd * m[b])[:, None] * k.T).astype(F8)  # [hw, ch]
        km8 = np.ascontiguousarray(
            km.reshape(hw // 256, 2, 128, ch).transpose(2, 0, 1, 3)
        ).reshape(128, 2 * hw)
        norm = (1.0 / rstd)
        for hh in range(2):
            cols = slice(hh * half, (hh + 1) * half)
            f8c = np.ascontiguousarray(f8_full[:, :, cols]).reshape(128, 2 * half)
            eb = np.full((128, 1), 5.0 - norm[cols].max(), dtype=np.float32)
            in_maps.append({
                "fs8": fs8, "f8": f8c, "km8": km8, "ebias": eb,
            })
    return in_maps


def kernel(foreground, mask):
    foreground = np.asarray(foreground, dtype=np.float32)
    mask = np.asarray(mask, dtype=np.float32)
    bs, ch, h, w = foreground.shape
    hw = h * w

    in_maps = make_in_maps(foreground, mask)
    try:
        results = _get_runner()(in_maps)
    except Exception:
        # robust fallback: the generic SPMD entry point
        res = run_bass_kernel_spmd(_get_program(), in_maps, list(range(8)))
        results = res.results

    fmap = np.empty((bs, ch, h, w), dtype=np.float32)
    rows = h // 2
    for core in range(8):
        b, hh = core // 2, core % 2
        o = results[core]["o"]       # [128, 2*hw/2] fp16 unnormalized
        s = results[core]["s"]       # [1, hw/2] softmax denominator
        o_f = o.astype(np.float32).reshape(128, 2, hw // 2)
        o_f = o_f.transpose(1, 0, 2).reshape(ch, hw // 2)
        fmap[b, :, hh * rows:(hh + 1) * rows, :] = (o_f / s).reshape(ch, rows, w)

    mm = mask[:, 0:1]                    # [bs, 1, h, w]
    final = fmap * (1.0 - mm) + foreground * mm
    skip = mask.sum(axis=(1, 2, 3)) > (hw - 10)
    final[skip] = foreground[skip]
    return final.astype(np.float32)
